# revision 51
# baseline (speedup 1.0000x reference)
"""Trainium2 Bass kernel for a teacher-forced GRU decoder with attention.

Model (per reference): each decode step's GRU starts from h=0 (the original
module never passes the hidden state), so all T steps are independent and the
whole computation vectorizes over (batch, time) rows:

    x      = embedding[dec_in]                        [B,T,E]
    gi     = x @ w_ih.T + b_ih
    r,z    = sigmoid(gi_r + b_hh_r), sigmoid(gi_z + b_hh_z)
    n      = tanh(gi_n + b_ih_n + r * b_hh_n)         (b_ih_n folded via bias)
    h      = (1-z) * n
    scores = h @ enc.T ; attn = softmax(scores) ; ctx = attn @ enc
    o      = tanh([h ctx] @ wa.T)
    logits = o @ fc_w.T + fc_b
    out    = log_softmax(logits)                      [B,T,V]
    hlast  = h[:, -1][None]                           [1,B,H]

Sharding: pure data parallel over batch — 8 batches per NeuronCore, no
collectives. On-device layout keeps activations transposed (feature dim on
partitions, rows on the free dim); at the two softmax points the transposed
activation serves directly as the matmul lhsT, flipping the result to
row-major exactly where free-dim reductions are needed.

log_softmax is computed without max subtraction (logits are ~1e-1, exp cannot
overflow): lse = ln(sum(exp(logits))), out = logits - lse.
"""

import os
import sys

import numpy as np

_TRN_REPO = "/opt/trn_rl_repo"
if _TRN_REPO not in sys.path:
    sys.path.insert(0, _TRN_REPO)

import ml_dtypes  # noqa: E402

import concourse.bass as bass  # noqa: E402
import concourse.mybir as mybir  # noqa: E402
import concourse.tile as tile  # noqa: E402
from concourse import bacc  # noqa: E402
from concourse.bass_utils import run_bass_kernel_spmd  # noqa: E402
from concourse.masks import make_identity  # noqa: E402

# Problem constants (hardcoded per task contract).
VOCAB, EMB, HID, SRC_LEN, BATCH, STEPS, SOS = 32000, 256, 512, 128, 64, 51, 1
NCORES = 8
BPC = BATCH // NCORES            # batches per core
R = BPC * STEPS                  # rows per core = 408
RT = [128, 128, 128, R - 384]    # row tiles (last = 24)
NMT = len(RT)                    # 4 M-tiles
KE = EMB // 128                  # 2 k-tiles for embedding dim
KH = HID // 128                  # 4 k-tiles for hidden dim
G3 = 3 * HID // 128              # 12 gate-dim tiles
NCH = 64                         # vocab chunks
CW = VOCAB // NCH                # chunk width = 500 (fits one PSUM bank fp32)
NP2 = 32                         # chunk-pairs (each = 2 chunks in 2 psum banks)
PW = 2 * CW                      # pair width = 1000
NGR = 8                          # store groups per M-tile
GW = VOCAB // NGR                # group width = 4000
P2PG = 4                         # chunk-pairs per store group
FCW_SCALE = 64.0                 # net logit scale in PSUM: fc_w fp8*8, o fp8*8;
                                 # undone via exp's input scale + pass C's mult

F32 = mybir.dt.float32
BF16 = mybir.dt.bfloat16
I32 = mybir.dt.int32

_cache = {}

last_exec_time_ns = None


def _build_nc():
    nc = bacc.Bacc("TRN2")

    # Per-core external inputs (host-prepared layouts).
    idx_d = nc.dram_tensor("idx", [128, NMT], I32, kind="ExternalInput")
    emb_d = nc.dram_tensor("emb", [VOCAB, EMB], F32, kind="ExternalInput")
    wih_d = nc.dram_tensor("wih", [128, KE, 3 * HID], F32, kind="ExternalInput")
    wihr_d = nc.dram_tensor(
        "wihr", [128, KE, 3 * HID], mybir.dt.float32r, kind="ExternalInput"
    )
    brz_d = nc.dram_tensor("brz", [128, 8], F32, kind="ExternalInput")
    bni_d = nc.dram_tensor("bni", [128, KH], F32, kind="ExternalInput")
    bnh_d = nc.dram_tensor("bnh", [128, KH], F32, kind="ExternalInput")
    enc_d = nc.dram_tensor("enc", [128, BPC, HID], BF16, kind="ExternalInput")
    encT_d = nc.dram_tensor("encT", [128, BPC, KH, SRC_LEN], BF16, kind="ExternalInput")
    wa_d = nc.dram_tensor("wa", [128, 2 * KH, HID], BF16, kind="ExternalInput")
    FP8 = mybir.dt.float8e4
    fcw_d = nc.dram_tensor(
        "fcw", [NP2, 128, 2, 2, 2, 512], FP8, kind="ExternalInput"
    )
    fcb_d = nc.dram_tensor("fcb", [VOCAB], BF16, kind="ExternalInput")

    out_d = nc.dram_tensor("out", [R, VOCAB], F32, kind="ExternalOutput")
    hout_d = nc.dram_tensor("hout", [KH, 128, BPC], F32, kind="ExternalOutput")

    AF = mybir.ActivationFunctionType
    OP = mybir.AluOpType

    with tile.TileContext(nc) as tc:
        from contextlib import ExitStack

        with ExitStack() as ctx:
            const = ctx.enter_context(tc.tile_pool(name="const", bufs=1))
            ident_f = const.tile([128, 128], F32)
            make_identity(nc, ident_f[:])
            ident_b = const.tile([128, 128], BF16)
            make_identity(nc, ident_b[:])
            ones_row = const.tile([128, 128], BF16)
            nc.gpsimd.memset(ones_row[:], 1.0)

            # Long-lived activation: o^T in fp8, K-pair-interleaved for the
            # DoubleRow FC matmul: o_dr[ki, kd, j, r] = 8*o_T[256*kd+2*ki+j, r]
            o_dr = const.tile([128, 2, 2, 416], mybir.dt.float8e4)

            # ---------------- Front end ----------------
            with ExitStack() as fctx:
                fpool = fctx.enter_context(tc.tile_pool(name="front", bufs=1))
                fps = fctx.enter_context(
                    tc.tile_pool(name="fpsum", bufs=2, space="PSUM")
                )
                aps = fctx.enter_context(
                    tc.tile_pool(name="apsum", bufs=4, space="PSUM")
                )
                fwork = fctx.enter_context(tc.tile_pool(name="fwork", bufs=3))

                idx_sb = fpool.tile([128, NMT], I32)
                nc.sync.dma_start(out=idx_sb[:], in_=idx_d[:])

                # Embedding gather first (on the critical path): row r of x
                # <- emb[dec_in[r]].
                x_rows = []
                for t in range(NMT):
                    xt = fpool.tile([128, EMB], F32, tag=f"xgather{t}")
                    nc.gpsimd.indirect_dma_start(
                        out=xt[:],
                        out_offset=None,
                        in_=emb_d[:],
                        in_offset=bass.IndirectOffsetOnAxis(
                            ap=idx_sb[:, t : t + 1], axis=0
                        ),
                    )
                    x_rows.append(xt)

                wih_sb = fpool.tile([128, KE, 3 * HID], F32)
                nc.sync.dma_start(out=wih_sb[:], in_=wih_d[:])
                wihr_sb = fpool.tile([128, KE, 3 * HID], mybir.dt.float32r)
                nc.sync.dma_start(out=wihr_sb[:], in_=wihr_d[:])
                brz_sb = fpool.tile([128, 8], F32)
                nc.sync.dma_start(out=brz_sb[:], in_=brz_d[:])
                bni_sb = fpool.tile([128, KH], F32)
                nc.sync.dma_start(out=bni_sb[:], in_=bni_d[:])
                bnh_sb = fpool.tile([128, KH], F32)
                nc.sync.dma_start(out=bnh_sb[:], in_=bnh_d[:])
                enc_sb = fpool.tile([128, BPC, HID], BF16)
                nc.sync.dma_start(out=enc_sb[:], in_=enc_d[:])
                encT_sb = fpool.tile([128, BPC, KH, SRC_LEN], BF16)
                nc.sync.dma_start(out=encT_sb[:], in_=encT_d[:])
                wa_sb = fpool.tile([128, 2 * KH, HID], BF16)
                nc.sync.dma_start(out=wa_sb[:], in_=wa_d[:])

                # Transpose x -> x^T [E, R].
                x_T = fpool.tile([128, KE, R], F32)
                for t in range(NMT):
                    for e in range(KE):
                        pst = fps.tile([128, 128], F32, tag="mm408")
                        nc.tensor.transpose(
                            out=pst[:],
                            in_=x_rows[t][:, e * 128 : (e + 1) * 128],
                            identity=ident_f[:],
                        )
                        nc.vector.tensor_copy(
                            out=x_T[:, e, t * 128 : t * 128 + RT[t]],
                            in_=pst[:, : RT[t]],
                        )

                # Bulk gi in fp32r (1 cyc/col; h is bf16 downstream so the
                # rounding is harmless); an exact fp32 duplicate for just the
                # t=T-1 columns produces decoder_hidden below.
                x_Tr = fpool.tile([128, KE, R], mybir.dt.float32r)
                nc.vector.tensor_copy(out=x_Tr[:], in_=x_T[:])
                r_t = fpool.tile([128, KH, R], F32)
                z_t = fpool.tile([128, KH, R], F32)
                n_t = fpool.tile([128, KH, R], F32)
                h_bf = fpool.tile([128, KH, R], BF16)
                for g in range(G3):
                    ps = fps.tile([128, R], F32, tag="mm408")
                    for k in range(KE):
                        nc.tensor.matmul(
                            ps[:],
                            lhsT=wihr_sb[:, k, g * 128 : (g + 1) * 128],
                            rhs=x_Tr[:, k, :],
                            start=(k == 0),
                            stop=(k == KE - 1),
                        )
                    if g < KH:  # r gates
                        nc.scalar.activation(
                            out=r_t[:, g, :],
                            in_=ps[:],
                            func=AF.Sigmoid,
                            bias=brz_sb[:, g : g + 1],
                        )
                    elif g < 2 * KH:  # z gates
                        nc.scalar.activation(
                            out=z_t[:, g - KH, :],
                            in_=ps[:],
                            func=AF.Sigmoid,
                            bias=brz_sb[:, g : g + 1],
                        )
                    else:  # n gates: tanh(gi + b_ih_n + r * b_hh_n)
                        q = g - 2 * KH
                        rb = fwork.tile([128, R], F32, tag="rb")
                        nc.vector.tensor_scalar(
                            out=rb[:],
                            in0=r_t[:, q, :],
                            scalar1=bnh_sb[:, q : q + 1],
                            scalar2=None,
                            op0=OP.mult,
                        )
                        s1 = fwork.tile([128, R], F32, tag="s1")
                        nc.vector.tensor_tensor(
                            out=s1[:], in0=ps[:], in1=rb[:], op=OP.add
                        )
                        nc.scalar.activation(
                            out=n_t[:, q, :],
                            in_=s1[:],
                            func=AF.Tanh,
                            bias=bni_sb[:, q : q + 1],
                        )
                for q in range(KH):
                    zn = fwork.tile([128, R], F32, tag="zn")
                    nc.vector.tensor_tensor(
                        out=zn[:], in0=z_t[:, q, :], in1=n_t[:, q, :], op=OP.mult
                    )
                    nc.vector.tensor_tensor(
                        out=h_bf[:, q, :], in0=n_t[:, q, :], in1=zn[:],
                        op=OP.subtract,
                    )

                # Exact fp32 duplicate of the GRU for the t=T-1 columns only
                # (8 rows) -> decoder_hidden at full fp32 precision.
                x_Ts = x_T[:, :, STEPS - 1 :: STEPS]          # [128, KE, BPC]
                rs_t = fwork.tile([128, KH, BPC], F32, tag="hs_r")
                hs_f = fpool.tile([128, KH, BPC], F32)
                for g in range(G3):
                    pss = aps.tile([128, BPC], F32, tag="attn", name="pss")
                    for k in range(KE):
                        nc.tensor.matmul(
                            pss[:],
                            lhsT=wih_sb[:, k, g * 128 : (g + 1) * 128],
                            rhs=x_Ts[:, k, :],
                            start=(k == 0),
                            stop=(k == KE - 1),
                        )
                    if g < KH:
                        nc.scalar.activation(
                            out=rs_t[:, g, :], in_=pss[:], func=AF.Sigmoid,
                            bias=brz_sb[:, g : g + 1],
                        )
                    elif g < 2 * KH:
                        qz = g - KH
                        zs = fwork.tile([128, BPC], F32, tag="hs_z")
                        nc.scalar.activation(
                            out=zs[:], in_=pss[:], func=AF.Sigmoid,
                            bias=brz_sb[:, g : g + 1],
                        )
                        # stash (1-z) = 1 - z into hs_f for later multiply
                        nc.vector.tensor_scalar(
                            out=hs_f[:, qz, :], in0=zs[:],
                            scalar1=-1.0, scalar2=1.0,
                            op0=OP.mult, op1=OP.add,
                        )
                    else:
                        qn = g - 2 * KH
                        rb2 = fwork.tile([128, BPC], F32, tag="hs_rb")
                        nc.vector.tensor_scalar(
                            out=rb2[:], in0=rs_t[:, qn, :],
                            scalar1=bnh_sb[:, qn : qn + 1], scalar2=None,
                            op0=OP.mult,
                        )
                        s2 = fwork.tile([128, BPC], F32, tag="hs_s")
                        nc.vector.tensor_tensor(
                            out=s2[:], in0=pss[:], in1=rb2[:], op=OP.add
                        )
                        ns = fwork.tile([128, BPC], F32, tag="hs_n")
                        nc.scalar.activation(
                            out=ns[:], in_=s2[:], func=AF.Tanh,
                            bias=bni_sb[:, qn : qn + 1],
                        )
                        nc.vector.tensor_tensor(
                            out=hs_f[:, qn, :], in0=hs_f[:, qn, :], in1=ns[:],
                            op=OP.mult,
                        )
                nc.sync.dma_start(
                    out=hout_d[:].rearrange("k p r -> p k r"), in_=hs_f[:]
                )

                # Attention per batch, then ctx^T tiles (cat k-tiles 4..7).
                ctx_bf = fpool.tile([128, KH, R], BF16)
                for b in range(BPC):
                    ps_sc = aps.tile([STEPS, SRC_LEN], F32, tag="attn")
                    for k in range(KH):
                        nc.tensor.matmul(
                            ps_sc[:],
                            lhsT=h_bf[:, k, b * STEPS : (b + 1) * STEPS],
                            rhs=encT_sb[:, b, k, :],
                            start=(k == 0),
                            stop=(k == KH - 1),
                        )
                    att = fwork.tile([STEPS, SRC_LEN], BF16, tag="att")
                    sm = fwork.tile([STEPS, 1], F32, tag="sm")
                    # scores are O(0.1): exp without max subtraction is safe.
                    nc.scalar.activation(
                        out=att[:], in_=ps_sc[:], func=AF.Exp, accum_out=sm[:]
                    )
                    rs = fwork.tile([STEPS, 1], F32, tag="rs")
                    nc.vector.reciprocal(out=rs[:], in_=sm[:])
                    nc.vector.tensor_scalar(
                        out=att[:],
                        in0=att[:],
                        scalar1=rs[:],
                        scalar2=None,
                        op0=OP.mult,
                    )
                    ps_at = aps.tile([SRC_LEN, STEPS], BF16, tag="attn")
                    nc.tensor.transpose(
                        out=ps_at[:], in_=att[:], identity=ident_b[:STEPS, :STEPS]
                    )
                    attT = fwork.tile([SRC_LEN, STEPS], BF16, tag="attTs")
                    nc.vector.tensor_copy(out=attT[:], in_=ps_at[:])
                    for k in range(KH):
                        ps_cx = aps.tile([128, STEPS], F32, tag="attn")
                        nc.tensor.matmul(
                            ps_cx[:],
                            lhsT=enc_sb[:, b, k * 128 : (k + 1) * 128],
                            rhs=attT[:],
                            start=True,
                            stop=True,
                        )
                        nc.vector.tensor_copy(
                            out=ctx_bf[:, k, b * STEPS : (b + 1) * STEPS],
                            in_=ps_cx[:],
                        )

                # o^T = tanh(wa @ cat^T), cat k-tiles = [h_bf, ctx_bf].
                # wa columns are host-permuted so psum rows come out in the
                # DoubleRow (ki, j) interleave; then scale by 8 into fp8.
                o_tanh = fpool.tile([128, 2, 2, 416], BF16)
                for kd in range(2):
                    ps_o = fps.tile([128, 2, 512], F32, tag="mm408", name="ps_o")
                    for j in range(2):
                        c = 2 * kd + j
                        for k in range(2 * KH):
                            src = h_bf[:, k, :] if k < KH else ctx_bf[:, k - KH, :]
                            nc.tensor.matmul(
                                ps_o[:, j, :R],
                                lhsT=wa_sb[:, k, c * 128 : (c + 1) * 128],
                                rhs=src,
                                start=(k == 0),
                                stop=(k == 2 * KH - 1),
                            )
                    nc.scalar.activation(
                        out=o_tanh[:, kd, :, :R],
                        in_=ps_o[:, :, :R],
                        func=AF.Tanh,
                    )
                    nc.vector.tensor_scalar(
                        out=o_dr[:, kd, :, :R],
                        in0=o_tanh[:, kd, :, :R],
                        scalar1=8.0,
                        scalar2=None,
                        op0=OP.mult,
                    )

            # ---------------- FC + log_softmax ----------------
            # M-tiles processed in pairs so fc_w streams only twice.
            # fc_b is broadcast-streamed per chunk (tiny SBUF footprint).
            with ExitStack() as cctx:
                fcw_pool = cctx.enter_context(tc.tile_pool(name="fcw", bufs=3))
                fcb_pool = cctx.enter_context(tc.tile_pool(name="fcbp", bufs=3))
                ps_pool = cctx.enter_context(
                    tc.tile_pool(name="cpsum", bufs=4, space="PSUM")
                )
                st_pool = cctx.enter_context(
                    tc.tile_pool(name="store", bufs=2 * NGR + 2)
                )
                ex_pool = cctx.enter_context(tc.tile_pool(name="expscr", bufs=2))
                bn_pool = cctx.enter_context(tc.tile_pool(name="bounce", bufs=3))
                sm_pool = cctx.enter_context(tc.tile_pool(name="sums", bufs=4))

                def emit_lse(m, sums_m):
                    """reduce partial sums -> -ln(sum); returns neglse tile"""
                    mw = RT[m]
                    tot = sm_pool.tile([128, 1], F32, tag="tot", name="tot")
                    nc.vector.reduce_sum(
                        out=tot[:mw], in_=sums_m[:mw, :], axis=mybir.AxisListType.X
                    )
                    neglse = sm_pool.tile([128, 1], F32, tag="neglse", name="nl")
                    nc.scalar.activation(out=neglse[:mw], in_=tot[:mw], func=AF.Ln)
                    nc.vector.tensor_scalar(
                        out=neglse[:mw],
                        in0=neglse[:mw],
                        scalar1=-1.0,
                        scalar2=None,
                        op0=OP.mult,
                    )
                    return neglse

                def emit_passc(m, g, stores_m, neglse):
                    """one group: out = store - lse -> f32 bounce -> HBM"""
                    mw = RT[m]
                    for hv in range(2):
                        hw2 = GW // 2
                        bt = bn_pool.tile([128, hw2], F32, tag="bounce", name="bt")
                        nc.vector.tensor_scalar(
                            out=bt[:mw],
                            in0=stores_m[g][:mw, hv * hw2 : (hv + 1) * hw2],
                            scalar1=1.0 / 64.0,
                            scalar2=neglse[:mw],
                            op0=OP.mult,
                            op1=OP.add,
                        )
                        col0 = g * GW + hv * hw2
                        # scalar-engine HWDGE ring: keeps the output stream
                        # off the SP ring that feeds fc_w (no HOL blocking)
                        nc.scalar.dma_start(
                            out=out_d[m * 128 : m * 128 + mw, col0 : col0 + hw2],
                            in_=bt[:mw],
                        )

                MGROUPS = [[0, 1], [2, 3]]
                # Global drain queue: (m, g, stores_m) units pending pass C.
                # Each phase drains a quota paced across its chunk loop,
                # keeping up to 8 units to hide in the phase after.
                pending = []
                neglse_of = {}
                prev = None  # (ms, stores, sums) of the previous M-group
                for mgi, ms in enumerate(MGROUPS):
                    stores = {}
                    sums = {}
                    for m in ms:
                        sums[m] = sm_pool.tile([128, NGR], F32, tag="sums", name="sums")
                        stores[m] = [
                            st_pool.tile([128, GW], BF16, tag="store", name="store")
                            for _ in range(NGR)
                        ]
                    last_phase = mgi == len(MGROUPS) - 1
                    quota = len(pending) if last_phase else max(0, len(pending) - 8)
                    drained = 0
                    for p2 in range(NP2):
                        g, j = p2 // P2PG, p2 % P2PG
                        fcw_t = fcw_pool.tile(
                            [128, 2, 2, 2, 512], mybir.dt.float8e4, tag="fcw"
                        )
                        nc.sync.dma_start(out=fcw_t[:], in_=fcw_d[p2])
                        fcb_s = fcb_pool.tile([128, PW], BF16, tag="fcbsrc")
                        nc.sync.dma_start(
                            out=fcb_s[:1, :],
                            in_=bass.AP(
                                tensor=fcb_d[:].tensor,
                                offset=p2 * PW,
                                ap=[[0, 1], [1, PW]],
                            ),
                        )
                        fcb_t = fcb_pool.tile([128, PW], BF16, tag="fcb")
                        nc.gpsimd.partition_broadcast(
                            out_ap=fcb_t[:], in_ap=fcb_s[:1, :]
                        )
                        for m in ms:
                            mw = RT[m]
                            # DVE is the binding engine of the matmul phase
                            # while ACT has a little headroom next to the exp
                            # passes: route ~1/8 of the psum drains to ACT
                            # (fc_b for those goes in via a rank-1 PE matmul).
                            act_drain = p2 % 8 == 3
                            # two psum banks per tile: [mw, 2, 512-padded]
                            ps = ps_pool.tile([mw, 2, 512], F32, tag="fc")
                            for q in range(2):
                                for kd in range(2):
                                    nc.tensor.matmul(
                                        ps[:, q, :CW],
                                        lhsT=o_dr[:, kd, :, m * 128 : m * 128 + mw],
                                        rhs=fcw_t[:, kd, q, :, :CW],
                                        start=(kd == 0),
                                        stop=(kd == 1 and not act_drain),
                                        perf_mode=mybir.MatmulPerfMode.DoubleRow,
                                    )
                                if act_drain:
                                    # += ones^T @ (64*fc_b) chunk
                                    nc.tensor.matmul(
                                        ps[:, q, :CW],
                                        lhsT=ones_row[:1, :mw],
                                        rhs=fcb_s[:1, q * CW : (q + 1) * CW],
                                        start=False,
                                        stop=True,
                                    )
                            if act_drain:
                                nc.scalar.activation(
                                    out=stores[m][g][
                                        :mw, j * PW : (j + 1) * PW
                                    ].rearrange("p (q v) -> p q v", q=2),
                                    in_=ps[:, :, :CW],
                                    func=AF.Copy,
                                )
                            else:
                                nc.vector.tensor_tensor(
                                    out=stores[m][g][
                                        :mw, j * PW : (j + 1) * PW
                                    ].rearrange("p (q v) -> p q v", q=2),
                                    in0=ps[:, :, :CW],
                                    in1=fcb_t[:mw, :].rearrange(
                                        "p (q v) -> p q v", q=2
                                    ),
                                    op=OP.add,
                                )
                        # emit exp for a finished group right away so ACT
                        # overlaps the matmul phase instead of tail-bursting
                        if j == P2PG - 1:
                            for m in ms:
                                mw = RT[m]
                                scr = ex_pool.tile(
                                    [128, GW], BF16, tag="expscr", name="scr"
                                )
                                nc.scalar.activation(
                                    out=scr[:mw],
                                    in_=stores[m][g][:mw],
                                    func=AF.Exp,
                                    scale=1.0 / 64.0,
                                    accum_out=sums[m][:mw, g : g + 1],
                                )
                        # interleave pending lse + pass C of earlier groups
                        # into this chunk loop (spreads the output DMA and
                        # frees store slots gradually)
                        if prev is not None and p2 == 0:
                            for m in prev[0]:
                                neglse_of[m] = emit_lse(m, prev[2][m])
                        target = quota * (p2 + 1) // NP2
                        while drained < target and pending:
                            m2_, g2, pst = pending.pop(0)
                            emit_passc(m2_, g2, pst, neglse_of[m2_])
                            drained += 1
                    for m in ms:
                        for g in range(NGR):
                            pending.append((m, g, stores[m]))
                    prev = (ms, stores, sums)

                # final tail: lse of the last group + whatever is pending
                for m in prev[0]:
                    neglse_of[m] = emit_lse(m, prev[2][m])
                for m2_, g2, pst in pending:
                    emit_passc(m2_, g2, pst, neglse_of[m2_])

    nc.compile()
    return nc


def _prep_shared(inputs):
    """Host-side layout prep for weights shared by all cores."""
    emb = np.ascontiguousarray(np.asarray(inputs["embedding"], dtype=np.float32))
    w_ih = np.asarray(inputs["w_ih"], dtype=np.float32)      # [3H, E]
    b_ih = np.asarray(inputs["b_ih"], dtype=np.float32)
    b_hh = np.asarray(inputs["b_hh"], dtype=np.float32)
    wa = np.asarray(inputs["wa"], dtype=np.float32)          # [H, 2H]
    fc_w = np.asarray(inputs["fc_w"], dtype=np.float32)      # [V, H]
    fc_b = np.asarray(inputs["fc_b"], dtype=np.float32)

    # w_ih^T [E, 3H] -> [128, KE, 3H]
    wih_t = np.ascontiguousarray(
        w_ih.T.reshape(KE, 128, 3 * HID).transpose(1, 0, 2)
    )
    brz = np.ascontiguousarray(
        (b_ih[: 2 * HID] + b_hh[: 2 * HID]).reshape(8, 128).T
    )
    bni = np.ascontiguousarray(b_ih[2 * HID :].reshape(KH, 128).T)
    bnh = np.ascontiguousarray(b_hh[2 * HID :].reshape(KH, 128).T)
    # wa^T [2H, H] -> [128, 2KH, H] bf16, output (hid) columns permuted
    # into the DoubleRow interleave: col (2*kd+j)*128+ki <- hid 256*kd+2*ki+j
    perm = np.empty(HID, np.int64)
    for kd in range(2):
        for j in range(2):
            for ki in range(128):
                perm[(2 * kd + j) * 128 + ki] = 256 * kd + 2 * ki + j
    wa_t = np.ascontiguousarray(
        wa.T[:, perm].reshape(2 * KH, 128, HID).transpose(1, 0, 2)
    ).astype(ml_dtypes.bfloat16)
    # fc_w^T [H, V] -> DoubleRow layout [NP2, 128(ki), 2(kd), 2(q), 2(j),
    # 512pad] fp8e4m3 scaled by 8 (o also x8 -> psum = 64*logit)
    a6 = (fc_w.T * 8.0).reshape(2, 128, 2, NP2, 2, CW)   # kd ki j p2 q v
    fcw = np.zeros((NP2, 128, 2, 2, 2, 512), ml_dtypes.float8_e4m3)
    fcw[..., :CW] = a6.transpose(3, 1, 0, 4, 2, 5).astype(ml_dtypes.float8_e4m3)
    fcb = (fc_b * 64.0).astype(ml_dtypes.bfloat16)
    return {
        "emb": emb,
        "wih": wih_t,
        "wihr": wih_t,
        "brz": brz,
        "bni": bni,
        "bnh": bnh,
        "wa": wa_t,
        "fcw": fcw,
        "fcb": fcb,
    }


def _prep_core(inputs, core):
    """Host-side prep of one core's batch shard."""
    target = np.asarray(inputs["target"])
    enc = np.asarray(inputs["encoder_outputs"], dtype=np.float32)
    bsl = slice(core * BPC, (core + 1) * BPC)
    tgt = target[bsl].astype(np.int32)                       # [BPC, T]
    dec_in = np.concatenate(
        [np.full((BPC, 1), SOS, np.int32), tgt[:, :-1]], axis=1
    ).reshape(-1)                                            # [R]
    idx = np.zeros(NMT * 128, np.int32)
    idx[:R] = dec_in
    idx = np.ascontiguousarray(idx.reshape(NMT, 128).T)      # [128, NMT]

    e = enc[bsl]                                             # [BPC, S, H]
    enc_sb = np.ascontiguousarray(e.transpose(1, 0, 2)).astype(
        ml_dtypes.bfloat16
    )                                                        # [S, BPC, H]
    encT = np.ascontiguousarray(
        e.reshape(BPC, SRC_LEN, KH, 128).transpose(3, 0, 2, 1)
    ).astype(ml_dtypes.bfloat16)                             # [128, BPC, KH, S]
    return {"idx": idx, "enc": enc_sb, "encT": encT}


def kernel(**inputs):
    global last_exec_time_ns
    if "nc" not in _cache:
        _cache["nc"] = _build_nc()
    nc = _cache["nc"]

    shared = _prep_shared(inputs)
    in_maps = []
    for core in range(NCORES):
        m = dict(shared)
        m.update(_prep_core(inputs, core))
        in_maps.append(m)

    trace = bool(int(os.environ.get("KERNEL_TRACE", "0")))
    res = run_bass_kernel_spmd(
        nc, in_maps, core_ids=list(range(NCORES)), trace=trace
    )
    last_exec_time_ns = res.exec_time_ns

    outs = res.results
    dec_out = np.concatenate(
        [outs[c]["out"].reshape(BPC, STEPS, VOCAB) for c in range(NCORES)],
        axis=0,
    )                                                        # [B, T, V]
    h_last = np.zeros((1, BATCH, HID), np.float32)
    for c in range(NCORES):
        hT = outs[c]["hout"].reshape(HID, BPC)               # [H, BPC]
        for lb in range(BPC):
            h_last[0, c * BPC + lb] = hT[:, lb]
    return dec_out, h_last
